# revision 7
# baseline (speedup 1.0000x reference)
"""Bass/Trainium2 kernel for a BiGRU encoder-decoder (B=1024, S=128, T=32, H=512, IN=3).

Sharding: data-parallel over batch across 8 NeuronCores (128 rows/core);
all weights replicated; sequential scans local to each core.

On-chip layout: feature-on-partitions. Activations are [feature, batch]
tiles; weights are the (pre-transposed, host-packed) stationary matmul
operand; gates accumulate in PSUM as [gate, batch] with biases folded in
via an augmented K=4 input ktile (x rows + ones row) or a K=1 ones ktile.
Matmul operands are fp16 (full PE rate), accumulation + gate math fp32.
"""

import os
import sys
import numpy as np
from contextlib import ExitStack

sys.path.insert(0, "/opt/trn_rl_repo")
import concourse.bass as bass  # noqa: E402,F401
import concourse.tile as tile  # noqa: E402
from concourse import bacc, mybir  # noqa: E402
from concourse.bass_utils import run_bass_kernel_spmd  # noqa: E402

FP16 = mybir.dt.float16
FP32 = mybir.dt.float32
AF = mybir.ActivationFunctionType

B, S, T, H, IN = 1024, 128, 32, 512, 3
NCORE = 8
BC = B // NCORE          # 128 batch rows per core
KT = H // 128            # 4 hidden k-tiles
GT = (3 * H) // 128      # 12 gate tiles (r 0-3, z 4-7, n 8-11)


# ----------------------------------------------------------------- host packing

def _pack_st(W, nk, ng):
    """Stationary pack of W [ng*128, nk*128] -> [128, nk*ng*128] fp16.

    Column block (k*ng + g)*128 holds lhsT for (ktile k, gtile g):
    lhsT[kf, gc] = W[g*128+gc, k*128+kf].
    """
    t = W.reshape(ng, 128, nk, 128).transpose(2, 3, 0, 1)  # [k, kf, g, gc]
    return np.ascontiguousarray(
        t.transpose(1, 0, 2, 3).reshape(128, nk * ng * 128)
    ).astype(np.float16)


def _pack_ihb(Wih, bih, bhh):
    """K=4 augmented input ktile: [x0,x1,x2,ones] rows -> 16 targets x 128 cols.

    Targets: r gtiles 0-3, z 4-7, n-input 8-11, n-hidden(bias only) 12-15.
    """
    out = np.zeros((4, 16 * 128), np.float32)
    for j in range(12):
        out[0:IN, j * 128:(j + 1) * 128] = Wih[j * 128:(j + 1) * 128, :].T
    bsum = bih + bhh
    for j in range(8):
        out[3, j * 128:(j + 1) * 128] = bsum[j * 128:(j + 1) * 128]
    for j in range(4):
        out[3, (8 + j) * 128:(9 + j) * 128] = bih[1024 + j * 128:1024 + (j + 1) * 128]
        out[3, (12 + j) * 128:(13 + j) * 128] = bhh[1024 + j * 128:1024 + (j + 1) * 128]
    return out.astype(np.float16)


def _pack_bias_row(bih, bhh):
    """K=1 ones-ktile bias row -> [1, 16*128] (same target order as _pack_ihb)."""
    row = np.zeros((1, 16 * 128), np.float32)
    row[0, 0:1024] = (bih + bhh)[0:1024]
    row[0, 1024:1536] = bih[1024:1536]
    row[0, 1536:2048] = bhh[1024:1536]
    return row.astype(np.float16)


def _host_pack(inp, s_steps, t_steps):
    """Build replicated weight arrays + per-core input arrays."""
    g = lambda k: np.asarray(inp[k], np.float32)
    w = {}
    for d, tag in ((0, "f"), (1, "b")):
        w[f"w_hh0{tag}"] = _pack_st(g("enc_Whh0")[d], KT, GT)
        w[f"w_ihb0{tag}"] = _pack_ihb(g("enc_Wih0")[d], g("enc_bih0")[d], g("enc_bhh0")[d])
        w[f"w_hh1{tag}"] = _pack_st(g("enc_Whh1")[d], KT, GT)
    # encoder layer 1 input gemm: K = 2H (fwd feat then bwd feat), G = 2*GT
    w_all = np.concatenate([g("enc_Wih1")[0], g("enc_Wih1")[1]], axis=0)  # [2*3H, 2H]
    w["w_ih1"] = _pack_st(w_all, 2 * KT, 2 * GT)
    bg = np.zeros((128, 2 * GT), np.float32)
    for d in range(2):
        bsum = g("enc_bih1")[d] + g("enc_bhh1")[d]
        for j in range(8):
            bg[:, d * GT + j] = bsum[j * 128:(j + 1) * 128]
        for j in range(4):
            bg[:, d * GT + 8 + j] = g("enc_bih1")[d][1024 + j * 128:1024 + (j + 1) * 128]
    w["b_gemm"] = bg
    nhb = np.zeros((1, 2 * 512), np.float32)
    for d in range(2):
        nhb[0, d * 512:(d + 1) * 512] = g("enc_bhh1")[d][1024:1536]
    w["w_nhb1"] = nhb.astype(np.float16)
    # decoder
    dihb0 = _pack_ihb(g("dec_Wih0"), g("dec_bih0"), g("dec_bhh0"))
    for r in range(3):
        w[f"w_dx{r}"] = np.ascontiguousarray(dihb0[r:r + 1])
    w["w_dxb"] = np.ascontiguousarray(dihb0[3:4])
    w["w_dhh"] = np.concatenate(
        [_pack_st(g("dec_Whh0"), KT, GT)]
        + [_pack_st(g("dec_Whh123")[i], KT, GT) for i in range(3)], axis=1)
    w["w_dih"] = np.concatenate(
        [_pack_st(g("dec_Wih123")[i], KT, GT) for i in range(3)], axis=1)
    w["w_dbias"] = np.concatenate(
        [_pack_bias_row(g("dec_bih123")[i], g("dec_bhh123")[i]) for i in range(3)],
        axis=1)
    lw = np.zeros((128, KT), np.float32)
    for k in range(KT):
        lw[:, k] = g("lin_W")[0, k * 128:(k + 1) * 128]
    w["w_lin"] = lw.astype(np.float16)
    lin_b = float(np.asarray(inp["lin_b"]).reshape(-1)[0])

    x = np.asarray(inp["x"], np.float32)  # [B, S, IN]
    per_core = []
    for c in range(NCORE):
        xc = x[c * BC:(c + 1) * BC]                      # [BC, S, IN]
        xt = xc.transpose(2, 1, 0)[:, :s_steps, :]       # [IN, s, BC]
        xa = np.concatenate([xt, np.ones((1, s_steps, BC), np.float32)], axis=0)
        m = dict(w)
        m["xa"] = np.ascontiguousarray(xa.reshape(4, s_steps * BC)).astype(np.float16)
        xlast = xc[:, s_steps - 1, :].T                  # [IN, BC]
        m["xd0"] = xlast.astype(np.float16)
        m["xd32"] = np.ascontiguousarray(xlast[0:2]).astype(np.float32)
        per_core.append(m)
    return per_core, lin_b


# ----------------------------------------------------------------- device build

def _emit_bank_group(nc, mms):
    """Emit matmuls as one psum-bank accumulation group."""
    last = len(mms) - 1
    for i, (out_ap, lhsT, rhs) in enumerate(mms):
        nc.tensor.matmul(out_ap, lhsT, rhs, start=(i == 0), stop=(i == last))


def _gru_gates(nc, gw, hp, g_tag, h_tag, pr, pz, pni, pnh, gi_sb, h32_prev):
    """Gate math for one GRU cell step. Returns (h32_new, h16_new).

    If gi_sb is not None (L1 scan): r/z pre-acts need +gi_sb[:, 0:1024] and the
    n-gate input part is gi_sb[:, 1024:1536]; otherwise pni psum holds it.
    """
    if gi_sb is not None:
        ar = gw.tile([128, 512], FP32, tag=f"ar{g_tag}")
        az = gw.tile([128, 512], FP32, tag=f"az{g_tag}")
        nc.vector.tensor_add(ar[:], pr[:], gi_sb[:, 0:512])
        nc.vector.tensor_add(az[:], pz[:], gi_sb[:, 512:1024])
        r_in, z_in = ar, az
    else:
        r_in, z_in = pr, pz
    r16 = gw.tile([128, 512], FP16, tag=f"r{g_tag}")
    z16 = gw.tile([128, 512], FP16, tag=f"z{g_tag}")
    nc.scalar.activation(r16[:], r_in[:], AF.Sigmoid)
    nc.scalar.activation(z16[:], z_in[:], AF.Sigmoid)
    t2 = gw.tile([128, 512], FP32, tag=f"t2{g_tag}")
    nc.vector.tensor_mul(t2[:], r16[:], pnh[:])
    t3 = gw.tile([128, 512], FP32, tag=f"t3{g_tag}")
    if gi_sb is None:
        nc.vector.tensor_add(t3[:], t2[:], pni[:])
    else:
        nc.vector.tensor_add(t3[:], t2[:], gi_sb[:, 1024:1536])
    n32 = gw.tile([128, 512], FP32, tag=f"n{g_tag}")
    nc.scalar.activation(n32[:], t3[:], AF.Tanh)
    d32 = gw.tile([128, 512], FP32, tag=f"d{g_tag}")
    nc.gpsimd.tensor_sub(d32[:], h32_prev[:], n32[:])
    e32 = gw.tile([128, 512], FP32, tag=f"e{g_tag}")
    nc.vector.tensor_mul(e32[:], d32[:], z16[:])
    h32n = hp.tile([128, 512], FP32, tag=f"h32{h_tag}")
    nc.gpsimd.tensor_add(h32n[:], n32[:], e32[:])
    h16n = hp.tile([128, 512], FP16, tag=f"h16{h_tag}")
    nc.vector.tensor_copy(h16n[:], h32n[:])
    return h32n, h16n


def build_nc(s_steps=S, t_steps=T, lin_b=0.0):
    nc = bacc.Bacc("TRN2", target_bir_lowering=False, debug=False,
                   num_devices=NCORE)

    dram_in = {}
    for name, shape, dt in [
        ("w_hh0f", [128, KT * GT * 128], FP16), ("w_hh0b", [128, KT * GT * 128], FP16),
        ("w_ihb0f", [4, 16 * 128], FP16), ("w_ihb0b", [4, 16 * 128], FP16),
        ("w_hh1f", [128, KT * GT * 128], FP16), ("w_hh1b", [128, KT * GT * 128], FP16),
        ("w_ih1", [128, 2 * KT * 2 * GT * 128], FP16),
        ("b_gemm", [128, 2 * GT], FP32),
        ("w_nhb1", [1, 1024], FP16),
        ("w_dx0", [1, 16 * 128], FP16), ("w_dx1", [1, 16 * 128], FP16),
        ("w_dx2", [1, 16 * 128], FP16), ("w_dxb", [1, 16 * 128], FP16),
        ("w_dhh", [128, 4 * KT * GT * 128], FP16),
        ("w_dih", [128, 3 * KT * GT * 128], FP16),
        ("w_dbias", [1, 3 * 16 * 128], FP16),
        ("w_lin", [128, KT], FP16),
        ("xa", [4, s_steps * BC], FP16),
        ("xd0", [3, BC], FP16),
        ("xd32", [2, BC], FP32),
    ]:
        dram_in[name] = nc.dram_tensor(name, shape, dt, kind="ExternalInput").ap()

    y_dram = nc.dram_tensor("y", [1, t_steps * BC], FP32, kind="ExternalOutput").ap()

    NCH = s_steps // 4  # gemm chunks (4 timesteps each)

    with tile.TileContext(nc) as tc, ExitStack() as ctx:
        const = ctx.enter_context(tc.tile_pool(name="const", bufs=1))
        hp = ctx.enter_context(tc.tile_pool(name="hstate", bufs=2))
        dram = ctx.enter_context(tc.tile_pool(name="dram", bufs=1, space="DRAM"))

        ones16 = const.tile([1, BC], FP16, tag="ones16")
        nc.vector.memset(ones16[:], 1.0)
        linb_sb = const.tile([1, 1], FP32, tag="linb_sb")
        nc.vector.memset(linb_sb[:], float(lin_b))
        y_sb = const.tile([1, t_steps * BC], FP32, tag="y_sb")

        out0_d = [dram.tile([s_steps, 128, KT * 128], FP16, tag=f"out0_{d}",
                            name=f"out0_{d}") for d in range(2)]
        gi1_d = [dram.tile([s_steps, 128, GT * 128], FP16, tag=f"gi1_{d}",
                           name=f"gi1_{d}") for d in range(2)]

        def load_const(name, pool):
            ap = dram_in[name]
            t = pool.tile(list(ap.shape), ap.dtype, tag=name)
            nc.sync.dma_start(t[:], ap[:])
            return t

        # =================== phase 0: encoder layer-0 bidirectional scan
        with tc.tile_pool(name="p0w", bufs=1) as p0w, \
             tc.tile_pool(name="gw0", bufs=2) as gw0, \
             tc.tile_pool(name="psum0", bufs=1, space="PSUM") as ps0:
            whh0 = [load_const("w_hh0f", p0w), load_const("w_hh0b", p0w)]
            wihb0 = [load_const("w_ihb0f", p0w), load_const("w_ihb0b", p0w)]
            xa = load_const("xa", p0w)

            h32 = [None, None]
            h16 = [None, None]
            for d in range(2):
                h32[d] = hp.tile([128, 512], FP32, tag=f"h32e0{d}", name=f"h32e0{d}")
                h16[d] = hp.tile([128, 512], FP16, tag=f"h16e0{d}", name=f"h16e0{d}")
                nc.vector.memset(h32[d][:], 0.0)
                nc.vector.memset(h16[d][:], 0.0)

            for step in range(s_steps):
                for d in range(2):
                    tt = step if d == 0 else s_steps - 1 - step
                    xmv = xa[:, tt * BC:(tt + 1) * BC]
                    pr = ps0.tile([128, 512], FP32, tag=f"pr{d}")
                    pz = ps0.tile([128, 512], FP32, tag=f"pz{d}")
                    pni = ps0.tile([128, 512], FP32, tag=f"pni{d}")
                    pnh = ps0.tile([128, 512], FP32, tag=f"pnh{d}")
                    wh, wi = whh0[d], wihb0[d]

                    def st_hh(k, g, wh=wh):
                        return wh[:, (k * GT + g) * 128:(k * GT + g + 1) * 128]

                    def st_ih(j, wi=wi):
                        return wi[:, j * 128:(j + 1) * 128]

                    for bank, gs, ihjs in ((pr, range(0, 4), range(0, 4)),
                                           (pz, range(4, 8), range(4, 8)),
                                           (pnh, range(8, 12), range(12, 16))):
                        mms = []
                        for s_i, (gg, j) in enumerate(zip(gs, ihjs)):
                            o = bank[:, s_i * 128:(s_i + 1) * 128]
                            for k in range(KT):
                                mms.append((o, st_hh(k, gg), h16[d][:, k * 128:(k + 1) * 128]))
                            mms.append((o, st_ih(j), xmv))
                        _emit_bank_group(nc, mms)
                    _emit_bank_group(nc, [
                        (pni[:, i * 128:(i + 1) * 128], st_ih(8 + i), xmv)
                        for i in range(4)])

                    h32[d], h16[d] = _gru_gates(
                        nc, gw0, hp, f"e{d}", f"e0{d}", pr, pz, pni, pnh, None, h32[d])
                    nc.sync.dma_start(out0_d[d][tt], h16[d][:])
            h_dec0 = [(h32[0], h16[0]), (h32[1], h16[1])]  # (h0f, h0b)

        # =================== phase 1: in1 @ Wih1.T big gemm (4 timesteps/chunk)
        with tc.tile_pool(name="p1w", bufs=1) as p1w, \
             tc.tile_pool(name="gmv", bufs=3) as gmv, \
             tc.tile_pool(name="gst", bufs=6) as gst, \
             tc.tile_pool(name="psumg", bufs=8, space="PSUM") as psg:
            wih1 = load_const("w_ih1", p1w)
            bgem = load_const("b_gemm", p1w)

            order = []
            lo, hi = 0, NCH - 1
            while lo <= hi:
                order.append(lo)
                if hi != lo:
                    order.append(hi)
                lo, hi = lo + 1, hi - 1

            for c in order:
                mv = []
                for d in range(2):
                    for j in range(KT):
                        m = gmv.tile([128, 512], FP16, tag=f"mv{d}{j}")
                        src = out0_d[d][4 * c:4 * c + 4, :, j * 128:(j + 1) * 128]
                        nc.sync.dma_start(
                            m[:].rearrange("p (t b) -> p t b", t=4),
                            src.rearrange("t p b -> p t b"))
                        mv.append(m)
                for G in range(2 * GT):
                    q = psg.tile([128, 512], FP32, tag="gq")
                    _emit_bank_group(nc, [
                        (q[:], wih1[:, (k * 2 * GT + G) * 128:(k * 2 * GT + G + 1) * 128],
                         mv[k][:])
                        for k in range(2 * KT)])
                    gsb = gst.tile([128, 512], FP16, tag="gsb")
                    nc.scalar.activation(gsb[:], q[:], AF.Identity,
                                         bias=bgem[:, G:G + 1])
                    d_, gl = (0, G) if G < GT else (1, G - GT)
                    dst = gi1_d[d_][4 * c:4 * c + 4, :, gl * 128:(gl + 1) * 128]
                    nc.sync.dma_start(dst.rearrange("t p b -> p t b"),
                                      gsb[:].rearrange("p (t b) -> p t b", t=4))

        # =================== phase 2: encoder layer-1 bidirectional scan
        with tc.tile_pool(name="p2w", bufs=1) as p2w, \
             tc.tile_pool(name="gw2", bufs=2) as gw2, \
             tc.tile_pool(name="gild", bufs=3) as gild, \
             tc.tile_pool(name="psum1", bufs=1, space="PSUM") as ps1:
            whh1 = [load_const("w_hh1f", p2w), load_const("w_hh1b", p2w)]
            nhb1 = load_const("w_nhb1", p2w)

            h32 = [None, None]
            h16 = [None, None]
            for d in range(2):
                h32[d] = hp.tile([128, 512], FP32, tag=f"h32e1{d}", name=f"h32e1{d}")
                h16[d] = hp.tile([128, 512], FP16, tag=f"h16e1{d}", name=f"h16e1{d}")
                nc.vector.memset(h32[d][:], 0.0)
                nc.vector.memset(h16[d][:], 0.0)

            for step in range(s_steps):
                for d in range(2):
                    tt = step if d == 0 else s_steps - 1 - step
                    gld = gild.tile([128, GT * 128], FP16, tag=f"gi{d}")
                    nc.sync.dma_start(gld[:], gi1_d[d][tt])
                    pr = ps1.tile([128, 512], FP32, tag=f"pr{d}")
                    pz = ps1.tile([128, 512], FP32, tag=f"pz{d}")
                    pnh = ps1.tile([128, 512], FP32, tag=f"pnh{d}")
                    wh = whh1[d]

                    def st_hh(k, g, wh=wh):
                        return wh[:, (k * GT + g) * 128:(k * GT + g + 1) * 128]

                    for bank, gs in ((pr, range(0, 4)), (pz, range(4, 8))):
                        mms = []
                        for s_i, gg in enumerate(gs):
                            o = bank[:, s_i * 128:(s_i + 1) * 128]
                            for k in range(KT):
                                mms.append((o, st_hh(k, gg), h16[d][:, k * 128:(k + 1) * 128]))
                        _emit_bank_group(nc, mms)
                    mms = []
                    for s_i, gg in enumerate(range(8, 12)):
                        o = pnh[:, s_i * 128:(s_i + 1) * 128]
                        for k in range(KT):
                            mms.append((o, st_hh(k, gg), h16[d][:, k * 128:(k + 1) * 128]))
                        mms.append((o, nhb1[:, d * 512 + s_i * 128:d * 512 + (s_i + 1) * 128],
                                    ones16[:]))
                    _emit_bank_group(nc, mms)

                    h32[d], h16[d] = _gru_gates(
                        nc, gw2, hp, f"f{d}", f"e1{d}", pr, pz, None, pnh, gld, h32[d])
            h_dec1 = [(h32[0], h16[0]), (h32[1], h16[1])]  # (h1f, h1b)

        # =================== phase 3: 4-layer decoder, autoregressive
        with tc.tile_pool(name="p3w", bufs=1) as p3w, \
             tc.tile_pool(name="gw3", bufs=2) as gw3, \
             tc.tile_pool(name="dx", bufs=2) as dx, \
             tc.tile_pool(name="psumd", bufs=1, space="PSUM") as psd:
            wdhh = load_const("w_dhh", p3w)
            wdih = load_const("w_dih", p3w)
            wdbias = load_const("w_dbias", p3w)
            wdx = [load_const("w_dx0", p3w), load_const("w_dx1", p3w),
                   load_const("w_dx2", p3w), load_const("w_dxb", p3w)]
            wlin = load_const("w_lin", p3w)

            xrow = []
            for r in range(3):
                xr = dx.tile([1, BC], FP16, tag=f"xr{r}", name=f"xr{r}")
                nc.sync.dma_start(xr[:], dram_in["xd0"][r:r + 1, :])
                xrow.append(xr)
            din = []
            for r in range(2):
                dr = dx.tile([1, BC], FP32, tag=f"din{r}", name=f"din{r}")
                nc.sync.dma_start(dr[:], dram_in["xd32"][r:r + 1, :])
                din.append(dr)

            hL = [h_dec0[0], h_dec0[1], h_dec1[0], h_dec1[1]]

            for t in range(t_steps):
                below16 = None
                for L in range(4):
                    h32p, h16p = hL[L]
                    pr = psd.tile([128, 512], FP32, tag="pr")
                    pz = psd.tile([128, 512], FP32, tag="pz")
                    pni = psd.tile([128, 512], FP32, tag="pni")
                    pnh = psd.tile([128, 512], FP32, tag="pnh")

                    def st_hh(k, g, L=L):
                        c0 = (L * KT + k) * GT + g
                        return wdhh[:, c0 * 128:(c0 + 1) * 128]

                    def st_ih0(r, j):
                        return wdx[r][:, j * 128:(j + 1) * 128]

                    def st_ih(k, g, L=L):
                        c0 = ((L - 1) * KT + k) * GT + g
                        return wdih[:, c0 * 128:(c0 + 1) * 128]

                    def st_b(j, L=L):
                        c0 = (L - 1) * 16 + j
                        return wdbias[:, c0 * 128:(c0 + 1) * 128]

                    for bank, gs, jbase in ((pr, range(0, 4), 0),
                                            (pz, range(4, 8), 4),
                                            (pnh, range(8, 12), 12)):
                        mms = []
                        for s_i, gg in enumerate(gs):
                            o = bank[:, s_i * 128:(s_i + 1) * 128]
                            for k in range(KT):
                                mms.append((o, st_hh(k, gg), h16p[:, k * 128:(k + 1) * 128]))
                            j = jbase + s_i
                            if L == 0:
                                for r in range(3):
                                    mms.append((o, st_ih0(r, j), xrow[r][:]))
                                mms.append((o, st_ih0(3, j), ones16[:]))
                            else:
                                if jbase != 12:  # r/z get ih contributions
                                    for k in range(KT):
                                        mms.append((o, st_ih(k, gg),
                                                    below16[:, k * 128:(k + 1) * 128]))
                                mms.append((o, st_b(j), ones16[:]))
                        _emit_bank_group(nc, mms)
                    mms = []
                    for s_i in range(4):
                        o = pni[:, s_i * 128:(s_i + 1) * 128]
                        if L == 0:
                            for r in range(3):
                                mms.append((o, st_ih0(r, 8 + s_i), xrow[r][:]))
                            mms.append((o, st_ih0(3, 8 + s_i), ones16[:]))
                        else:
                            for k in range(KT):
                                mms.append((o, st_ih(k, 8 + s_i),
                                            below16[:, k * 128:(k + 1) * 128]))
                            mms.append((o, st_b(8 + s_i), ones16[:]))
                    _emit_bank_group(nc, mms)

                    h32n, h16n = _gru_gates(
                        nc, gw3, hp, "dd", f"d{L}", pr, pz, pni, pnh, None, h32p)
                    hL[L] = (h32n, h16n)
                    below16 = h16n

                # output projection + feedback
                pout = psd.tile([1, BC], FP32, tag="pni")
                _emit_bank_group(nc, [
                    (pout[:], wlin[:, k:k + 1], below16[:, k * 128:(k + 1) * 128])
                    for k in range(KT)])
                ysl = y_sb[:, t * BC:(t + 1) * BC]
                nc.scalar.activation(ysl, pout[:], AF.Identity, bias=linb_sb[:])
                if t + 1 < t_steps:
                    n1 = dx.tile([1, BC], FP32, tag="n1")
                    nc.vector.tensor_sub(n1[:], din[0][:], ysl)
                    n2 = dx.tile([1, BC], FP32, tag="n2")
                    nc.vector.tensor_sub(n2[:], din[1][:], n1[:])
                    d0n = dx.tile([1, BC], FP32, tag="din0")
                    nc.vector.tensor_copy(d0n[:], ysl)
                    xr0n = dx.tile([1, BC], FP16, tag="xr0")
                    nc.vector.tensor_copy(xr0n[:], ysl)
                    xr1n = dx.tile([1, BC], FP16, tag="xr1")
                    nc.vector.tensor_copy(xr1n[:], n1[:])
                    xr2n = dx.tile([1, BC], FP16, tag="xr2")
                    nc.vector.tensor_copy(xr2n[:], n2[:])
                    xrow = [xr0n, xr1n, xr2n]
                    din = [d0n, n1]

            nc.sync.dma_start(y_dram[:], y_sb[:])

    nc.compile()
    return nc


# ----------------------------------------------------------------- entry point

def kernel(**inputs) -> np.ndarray:
    s_steps = int(os.environ.get("BGRU_S", S))
    t_steps = int(os.environ.get("BGRU_T", T))
    per_core, lin_b = _host_pack(inputs, s_steps, t_steps)
    nc = build_nc(s_steps, t_steps, lin_b)
    res = run_bass_kernel_spmd(nc, per_core, list(range(NCORE)))
    out = np.zeros((B, t_steps, 1), np.float32)
    for c in range(NCORE):
        yc = res.results[c]["y"].reshape(t_steps, BC)  # [t, b]
        out[c * BC:(c + 1) * BC, :, 0] = yc.T
    return out
